# revision 7
# baseline (speedup 1.0000x reference)
"""GQA attention (B=2,S=2048,DIM=4096, 32 Q heads / 8 KV heads, HD=128, RoPE,
full non-causal softmax, output projection) on 8 trn2 NeuronCores.

Sharding: token-parallel. Each core owns 512 token rows (cores 0-3 = batch 0,
4-7 = batch 1). Per core: project Q/K/V for its tokens, RoPE Q/K, AllGather
K/V within the 4-core batch group (2x2MB, cheap), run all 32 heads of
attention for its 512 queries over the full 2048 keys, then the full wo
matmul for its token slice. Outputs are disjoint across cores -> host concat,
no AllReduce.

Layout tricks (all chosen to avoid on-device transposes):
  - x is transposed on host to xT [DIM, TOK]; every matmul then has its
    contraction dim on partitions with natural slicing.
  - Q/K are kept feature-major [feat, tok]; V token-major [tok, feat].
  - scores are computed transposed (keys on partitions) so softmax's
    denominator is a PE column-sum via an all-ones stationary matrix
    (broadcast to 128 partitions for free) and exp is a single ACT pass
    with the 1/sqrt(HD) scale folded in.
  - RoPE pairs are split into rotate-half layout by permuting wq/wk columns
    on host, so on-device RoPE is 2 ACT copies + 3 full-height DVE ops.
  - ctx bounces through DRAM between attention and wo to keep SBUF under
    the per-partition budget (stack allocator).
"""

import numpy as np

import concourse.bacc as bacc
import concourse.mybir as mybir
import concourse.tile as tile
from concourse.bass_utils import run_bass_kernel_spmd

B, S, DIM = 2, 2048, 4096
NH, NKV, HD = 32, 8, 128
NREP = NH // NKV
NCORES = 8
GROUP = 4                 # cores per batch group
TOK = B * S               # 4096
TOKL = TOK // NCORES      # 512 tokens per core
P = 128
DIMO = DIM // P           # 32 contraction chunks
KC = S // P               # 16 key chunks per batch
SCALE = 1.0 / float(np.sqrt(HD))
F32 = mybir.dt.float32

_CACHED = {}


def _build():
    nc = bacc.Bacc(trn_type="TRN2", num_devices=NCORES, debug=False)

    xT = nc.dram_tensor("xT", [DIM, TOKL], F32, kind="ExternalInput")
    wq = nc.dram_tensor("wq", [DIM, NH * HD], F32, kind="ExternalInput")
    wk = nc.dram_tensor("wk", [DIM, NKV * HD], F32, kind="ExternalInput")
    wv = nc.dram_tensor("wv", [DIM, NKV * HD], F32, kind="ExternalInput")
    wo = nc.dram_tensor("wo", [DIM, DIM], F32, kind="ExternalInput")
    cos = nc.dram_tensor("cos", [HD // 2, TOKL], F32, kind="ExternalInput")
    sin = nc.dram_tensor("sin", [HD // 2, TOKL], F32, kind="ExternalInput")
    out = nc.dram_tensor("out", [TOKL, DIM], F32, kind="ExternalOutput")
    k_out = nc.dram_tensor("k_out", [NKV * HD, TOKL], F32, kind="ExternalOutput")
    v_out = nc.dram_tensor("v_out", [TOKL, NKV * HD], F32, kind="ExternalOutput")

    rg = [[0, 1, 2, 3], [4, 5, 6, 7]]

    with tile.TileContext(nc) as tc:
        with (
            tc.tile_pool(name="dram", bufs=1, space="DRAM") as dram,
            tc.tile_pool(name="const", bufs=1) as const,
        ):
            kT_loc = dram.tile([NKV * HD, TOKL], F32)
            v_loc = dram.tile([TOKL, NKV * HD], F32)
            kT_all = dram.tile([GROUP * NKV * HD, TOKL], F32)
            v_all = dram.tile([GROUP * TOKL, NKV * HD], F32)
            ctx_dram = dram.tile([NH * HD, TOKL], F32)

            ones_sb = const.tile([P, P], F32)
            nc.vector.memset(ones_sb[:], 1.0)
            cosD = const.tile([P, TOKL], F32)   # cos duplicated on both halves
            sinN = const.tile([P, TOKL], F32)   # [-sin; +sin]
            nc.sync.dma_start(cosD[0:64, :], cos[:])
            nc.sync.dma_start(cosD[64:128, :], cos[:])
            nc.sync.dma_start(sinN[0:64, :], sin[:])
            nc.sync.dma_start(sinN[64:128, :], sin[:])
            nc.vector.tensor_scalar_mul(sinN[0:64, :], sinN[0:64, :], -1.0)

            def rope(ps, dst, swp_pool, tmp_pool):
                # dst = rotate_half RoPE of ps ([P, TOKL] psum).
                swp = swp_pool.tile([P, TOKL], F32, tag="rope_swp", name="swp")
                nc.scalar.copy(swp[0:64, :], ps[64:128, :])
                nc.scalar.copy(swp[64:128, :], ps[0:64, :])
                tmp = tmp_pool.tile([P, TOKL], F32, tag="rope_tmp", name="tmp")
                nc.vector.tensor_tensor(tmp[:], swp[:], sinN[:], mybir.AluOpType.mult)
                nc.vector.tensor_tensor(dst, ps[:], cosD[:], mybir.AluOpType.mult)
                nc.vector.tensor_tensor(dst, dst, tmp[:], mybir.AluOpType.add)

            # qT pool outlives xT (stack allocator: open first, free later)
            with tc.tile_pool(name="qtp", bufs=1) as qtp:
                qT_sb = qtp.tile([P, NH, TOKL], F32)

                with tc.tile_pool(name="xtp", bufs=1) as xtp:
                    xT_sb = xtp.tile([P, DIMO, TOKL], F32)
                    nc.sync.dma_start(
                        xT_sb[:], xT.ap().rearrange("(ko p) t -> p ko t", p=P)
                    )

                    # ---- K projection + RoPE ----
                    with (
                        tc.tile_pool(name="pp", bufs=3, space="PSUM") as pp,
                        tc.tile_pool(name="kvst", bufs=3) as kvst,
                        tc.tile_pool(name="rsw", bufs=2) as rsw,
                        tc.tile_pool(name="rtm", bufs=2) as rtm,
                        tc.tile_pool(name="wchk", bufs=3) as wchk,
                    ):
                        for g in range(NKV):
                            wkc = wchk.tile([P, DIMO, HD], F32, tag="wkvc", name="wkc")
                            nc.sync.dma_start(
                                wkc[:],
                                wk.ap()[:, g * HD : (g + 1) * HD].rearrange(
                                    "(ko p) f -> p ko f", p=P
                                ),
                            )
                            ps = pp.tile([P, TOKL], F32, tag="proj", name="psk")
                            for ko in range(DIMO):
                                nc.tensor.matmul(
                                    ps[:],
                                    lhsT=wkc[:, ko, :],
                                    rhs=xT_sb[:, ko, :],
                                    start=(ko == 0),
                                    stop=(ko == DIMO - 1),
                                )
                            kst = kvst.tile([P, TOKL], F32, tag="kst", name="kst")
                            rope(ps, kst[:], rsw, rtm)
                            nc.sync.dma_start(kT_loc[g * HD : (g + 1) * HD, :], kst[:])
                            nc.sync.dma_start(k_out[g * HD : (g + 1) * HD, :], kst[:])

                        # ---- V projection (token-major), wv in 128-col chunks ----
                        VF = 128
                        for fb in range((NKV * HD) // VF):
                            wvc = wchk.tile([P, DIMO, VF], F32, tag="wkvc", name="wvc")
                            nc.sync.dma_start(
                                wvc[:],
                                wv.ap()[:, fb * VF : (fb + 1) * VF].rearrange(
                                    "(ko p) f -> p ko f", p=P
                                ),
                            )
                            for t4 in range(TOKL // P):
                                ps = pp.tile([P, VF], F32, tag="projv", name="psv")
                                for ko in range(DIMO):
                                    nc.tensor.matmul(
                                        ps[:],
                                        lhsT=xT_sb[:, ko, t4 * P : (t4 + 1) * P],
                                        rhs=wvc[:, ko, :],
                                        start=(ko == 0),
                                        stop=(ko == DIMO - 1),
                                    )
                                vst = kvst.tile([P, VF], F32, tag="vst", name="vst")
                                nc.scalar.copy(vst[:], ps[:])
                                nc.sync.dma_start(
                                    v_loc[t4 * P : (t4 + 1) * P, fb * VF : (fb + 1) * VF],
                                    vst[:],
                                )
                                nc.sync.dma_start(
                                    v_out[t4 * P : (t4 + 1) * P, fb * VF : (fb + 1) * VF],
                                    vst[:],
                                )

                        nc.gpsimd.collective_compute(
                            "AllGather",
                            mybir.AluOpType.bypass,
                            replica_groups=rg,
                            ins=[kT_loc[:].opt()],
                            outs=[kT_all[:].opt()],
                        )
                        nc.gpsimd.collective_compute(
                            "AllGather",
                            mybir.AluOpType.bypass,
                            replica_groups=rg,
                            ins=[v_loc[:].opt()],
                            outs=[v_all[:].opt()],
                        )

                    # ---- Q projection + RoPE ----
                    with (
                        tc.tile_pool(name="wqch", bufs=3) as wqch,
                        tc.tile_pool(name="qpp", bufs=3, space="PSUM") as qpp,
                        tc.tile_pool(name="qsw", bufs=2) as qsw,
                        tc.tile_pool(name="qtm", bufs=2) as qtm,
                    ):
                        for h in range(NH):
                            wqc = wqch.tile([P, DIMO, HD], F32, tag="wqc", name="wqc")
                            nc.sync.dma_start(
                                wqc[:],
                                wq.ap()[:, h * HD : (h + 1) * HD].rearrange(
                                    "(ko p) f -> p ko f", p=P
                                ),
                            )
                            ps = qpp.tile([P, TOKL], F32, tag="qproj", name="psq")
                            for ko in range(DIMO):
                                nc.tensor.matmul(
                                    ps[:],
                                    lhsT=wqc[:, ko, :],
                                    rhs=xT_sb[:, ko, :],
                                    start=(ko == 0),
                                    stop=(ko == DIMO - 1),
                                )
                            rope(ps, qT_sb[:, h, :], qsw, qtm)

                # xT freed here
                # ---- attention ----
                with (
                    tc.tile_pool(name="kg", bufs=2) as kgp,
                    tc.tile_pool(name="vg", bufs=2) as vgp,
                    tc.tile_pool(name="at", bufs=4) as atp,
                    tc.tile_pool(name="rec", bufs=2) as recp,
                    tc.tile_pool(name="cst", bufs=3) as cstp,
                    tc.tile_pool(name="scps", bufs=3, space="PSUM") as scps,
                    tc.tile_pool(name="ctxps", bufs=2, space="PSUM") as ctxps,
                    tc.tile_pool(name="denps", bufs=2, space="PSUM") as denps,
                ):
                    for g in range(NKV):
                        kg = kgp.tile([P, GROUP, TOKL], F32, tag="kg", name="kg")
                        for r in range(GROUP):
                            nc.sync.dma_start(
                                kg[:, r, :],
                                kT_all[(r * NKV + g) * HD : (r * NKV + g + 1) * HD, :],
                            )
                        vg = vgp.tile([P, KC, HD], F32, tag="vg", name="vg")
                        for r in range(GROUP):
                            nc.sync.dma_start(
                                vg[:, r * 4 : (r + 1) * 4, :],
                                v_all[
                                    r * TOKL : (r + 1) * TOKL, g * HD : (g + 1) * HD
                                ].rearrange("(kc p) f -> p kc f", p=P),
                            )
                        for h in range(g * NREP, (g + 1) * NREP):
                            ctxp = ctxps.tile([P, TOKL], F32, tag="ctx", name="ctxp")
                            denp = denps.tile([P, TOKL], F32, tag="den", name="denp")
                            for kc in range(KC):
                                sp = scps.tile([P, TOKL], F32, tag="sc", name="sp")
                                nc.tensor.matmul(
                                    sp[:],
                                    lhsT=kg[:, kc // 4, (kc % 4) * P : (kc % 4 + 1) * P],
                                    rhs=qT_sb[:, h, :],
                                    start=True,
                                    stop=True,
                                )
                                at = atp.tile([P, TOKL], F32, tag="at", name="at")
                                nc.scalar.activation(
                                    at[:],
                                    sp[:],
                                    mybir.ActivationFunctionType.Exp,
                                    scale=SCALE,
                                )
                                nc.tensor.matmul(
                                    ctxp[:],
                                    lhsT=vg[:, kc, :],
                                    rhs=at[:],
                                    start=(kc == 0),
                                    stop=(kc == KC - 1),
                                )
                                nc.tensor.matmul(
                                    denp[:],
                                    lhsT=ones_sb[:],
                                    rhs=at[:],
                                    start=(kc == 0),
                                    stop=(kc == KC - 1),
                                )
                            rec = recp.tile([P, TOKL], F32, tag="rec", name="rec")
                            nc.vector.reciprocal(rec[:], denp[:])
                            cst = cstp.tile([P, TOKL], F32, tag="cst", name="cst")
                            nc.vector.tensor_tensor(
                                cst[:], ctxp[:], rec[:], mybir.AluOpType.mult
                            )
                            nc.sync.dma_start(
                                ctx_dram[h * HD : (h + 1) * HD, :], cst[:]
                            )

            # qT freed here
            # ---- output projection (ctx reloaded from DRAM) ----
            with (
                tc.tile_pool(name="ctp", bufs=1) as ctp,
                tc.tile_pool(name="wop", bufs=3) as wop,
                tc.tile_pool(name="obp", bufs=4) as obp,
                tc.tile_pool(name="ops", bufs=1, space="PSUM") as ops,
            ):
                ctx_sb = ctp.tile([P, NH, TOKL], F32)
                nc.sync.dma_start(
                    ctx_sb[:], ctx_dram[:].rearrange("(fc p) t -> p fc t", p=P)
                )
                DQ = 1024
                for dq in range(DIM // DQ):
                    pst = [
                        [
                            ops.tile(
                                [P, 512],
                                F32,
                                tag=f"o{t4}_{db}",
                                name=f"ops_{dq}_{t4}_{db}",
                            )
                            for db in range(2)
                        ]
                        for t4 in range(TOKL // P)
                    ]
                    for fc in range(DIMO):
                        wob = wop.tile([P, DQ], F32, tag="wob", name="wob")
                        nc.sync.dma_start(
                            wob[:],
                            wo.ap()[fc * P : (fc + 1) * P, dq * DQ : (dq + 1) * DQ],
                        )
                        for t4 in range(TOKL // P):
                            for db in range(2):
                                nc.tensor.matmul(
                                    pst[t4][db][:],
                                    lhsT=ctx_sb[:, fc, t4 * P : (t4 + 1) * P],
                                    rhs=wob[:, db * 512 : (db + 1) * 512],
                                    start=(fc == 0),
                                    stop=(fc == DIMO - 1),
                                )
                    for t4 in range(TOKL // P):
                        ob = obp.tile([P, DQ], F32, tag="ob", name="ob")
                        nc.scalar.copy(ob[:, 0:512], pst[t4][0][:])
                        nc.scalar.copy(ob[:, 512:1024], pst[t4][1][:])
                        nc.sync.dma_start(
                            out[t4 * P : (t4 + 1) * P, dq * DQ : (dq + 1) * DQ],
                            ob[:],
                        )

    nc.finalize()
    return nc


_PERM = np.concatenate([np.arange(0, HD, 2), np.arange(1, HD, 2)])
_INV_PERM = np.argsort(_PERM)


def _get_nc():
    if "nc" not in _CACHED:
        _CACHED["nc"] = _build()
    return _CACHED["nc"]


def kernel(x, freqs_cis, wq, wk, wv, wo, _trace=False):
    x = np.ascontiguousarray(np.asarray(x, np.float32))
    freqs_cis = np.asarray(freqs_cis, np.float32)
    wq = np.asarray(wq, np.float32)
    wk = np.asarray(wk, np.float32)
    wv = np.ascontiguousarray(np.asarray(wv, np.float32))
    wo = np.ascontiguousarray(np.asarray(wo, np.float32))

    xT = np.ascontiguousarray(x.reshape(TOK, DIM).T)
    wq_p = np.ascontiguousarray(
        wq.reshape(DIM, NH, HD)[:, :, _PERM].reshape(DIM, NH * HD)
    )
    wk_p = np.ascontiguousarray(
        wk.reshape(DIM, NKV, HD)[:, :, _PERM].reshape(DIM, NKV * HD)
    )
    cosT = np.ascontiguousarray(freqs_cis[:, :, 0].T)  # [64, S]
    sinT = np.ascontiguousarray(freqs_cis[:, :, 1].T)

    in_maps = []
    for c in range(NCORES):
        t0 = c * TOKL
        s0 = t0 % S
        in_maps.append(
            {
                "xT": np.ascontiguousarray(xT[:, t0 : t0 + TOKL]),
                "wq": wq_p,
                "wk": wk_p,
                "wv": wv,
                "wo": wo,
                "cos": np.ascontiguousarray(cosT[:, s0 : s0 + TOKL]),
                "sin": np.ascontiguousarray(sinT[:, s0 : s0 + TOKL]),
            }
        )

    nc = _get_nc()
    res = run_bass_kernel_spmd(
        nc, in_maps, core_ids=list(range(NCORES)), trace=_trace
    )
    if _trace:
        _CACHED["last_result"] = res

    out = np.concatenate(
        [res.results[c]["out"] for c in range(NCORES)], axis=0
    ).reshape(B, S, DIM)
    # new_k: per-core [NKV*HD, TOKL] feature-major, rotate-half-permuted
    ks = []
    vs = []
    for c in range(NCORES):
        kc = res.results[c]["k_out"].reshape(NKV, HD, TOKL)[:, _INV_PERM, :]
        ks.append(np.transpose(kc, (2, 0, 1)))  # [TOKL, NKV, HD]
        vs.append(res.results[c]["v_out"].reshape(TOKL, NKV, HD))
    new_k = np.concatenate(ks, axis=0).reshape(B, S, NKV, HD)
    new_v = np.concatenate(vs, axis=0).reshape(B, S, NKV, HD)
    return out, new_k, new_v
